# revision 22
# baseline (speedup 1.0000x reference)
"""Trainium2 Bass kernel for masked causal multi-head attention.

Problem (hardcoded):
    x: (4, 2048, 512) f32, m: (4, 2048, 1) f32 (prefix 0/1 mask),
    w_qkv: (512, 1536) f32, w_out: (512, 512) f32, b_out: (512,) f32
    out = (softmax(mask(QK^T/8)) V) @ w_out + b_out, masked by m.

Sharding: 8 cores = 4 batches x 2 head-groups (4 heads each).  Each core
computes the qkv projection for its (batch, head-group), flash-style causal
attention, and a partial out-projection; the host sums the two partials
per batch and adds b_out.

Kernel structure (all compute bf16, accumulation f32 in PSUM; fp8 was
tried for P/V and rejected: quantization error ~2.5e-2 vs the 2e-2 gate,
because the output magnitude scales as 1/sqrt(N_eff) exactly like the
quantization noise, so there is no averaging benefit):
  - Q^T, K^T in (dh, t) layout, two heads stacked per 128 partitions ->
    scores computed transposed: S^T (k, q), so softmax needs no transposes.
    No max-subtraction: scores are ~N(0,1), |s| <= ~7; exp(s-4) is safe.
  - exp on the scalar engine (its ONLY job: exp column count matches S
    column count, so the scalar engine is the second-busiest after PE).
    Causal triangle is a post-exp multiply by a 0/1 triangle (gpsimd).
  - V in key-block-pair tiles with an extra all-ones 65th column per head
    (row-sum trick); O^T = V_aug^T P accumulated over key blocks in PSUM.
    Diagonal-pair second members are computed padded to the pair width;
    the invalid P region is zeroed by a gpsimd memset before AV.
  - 1/l is broadcast across partitions with gpsimd partition_broadcast
    (no PE involvement, unlike the old ones(1,64) matmul broadcast which
    cost 22us of PE); the per-head normalize multiplies are deferred
    into the out-projection quanta (deep slack, off every critical path).
  - qkv projection for superblock s+1 and out-projection for s-1 are cut
    into small quanta and woven between attention chunks so the PE stream
    never goes dry (the HAM clock gate halves the PE clock for ~10us after
    any idle window -- gaplessness is worth more than instruction count).
  - Engine assignment: Scalar=exp only; DVE=all PSUM readers (q/k/v
    drains, O drains, l-row extracts, reciprocals, outproj mask-scales);
    Pool=SBUF elementwise (triangle, normalize muls, pad/ones memsets,
    l-transpose DMA issues); Sync=bulk DMA issue.
  - All host-side tensors are packed so every weight is ONE contiguous
    dma_start (DGE config on an engine sequencer costs ~600ns; the old
    per-block DMAs serialized ~15us of config at startup).
"""

import sys
from collections import deque

import numpy as np

try:
    import concourse.bass as bass  # noqa: F401
except ImportError:  # pragma: no cover
    sys.path.insert(0, "/opt/trn_rl_repo")

import concourse.bacc as bacc
import concourse.mybir as mybir
import concourse.tile as tile
from concourse import bass_utils

F32 = mybir.dt.float32
BF16 = mybir.dt.bfloat16
F8 = mybir.dt.float8e4
NP_BF16 = mybir.dt.np(BF16)
AF = mybir.ActivationFunctionType
DR = mybir.MatmulPerfMode.DoubleRow

B, T, D, H = 4, 2048, 512, 8
DH = D // H  # 64
G = 2  # head groups (cores per batch)
SCALE = DH**-0.5
EXP_BIAS = -4.0  # exp(s-4): keeps P in fp8e4m3 range; cancels in softmax
N_CORES = 8


def plan_segs(s, nblk, cap=1024):
    """Segment plan for superblock s: DoubleRow pairs + trailing single.

    Returns (chunks, n_av) where chunks is a list of (segs, used) and each
    seg is a dict: off (col offset in the 1024-wide chunk tile), W (member
    width), qoff (query offset of the segment in the superblock), dr
    (DoubleRow pair or single), members [(kb, c0_tri or None, pad)].
    """
    L = nblk * 128
    F = min(512, L - 512 * s)
    KB = min(4 * s + (F + 127) // 128, nblk)
    segs_all = []
    kb = 0
    while kb < KB:
        qa = max(0, 128 * (kb - 4 * s))
        W = F - qa
        if kb + 1 < KB:
            members = []
            for j, kbx in enumerate((kb, kb + 1)):
                dq = 128 * (kbx - 4 * s)  # diag col within segment (abs)
                c0 = dq - qa if dq >= qa else None  # tri pos, None=off-diag
                pad = (dq - qa) if (j == 1 and dq > qa) else 0
                members.append((kbx, c0, pad))
            segs_all.append(dict(W=W, qoff=qa, dr=True, members=members))
            kb += 2
        else:
            dq = 128 * (kb - 4 * s)
            c0 = dq - qa if dq >= qa else None
            segs_all.append(
                dict(W=W, qoff=qa, dr=False, members=[(kb, c0, 0)])
            )
            kb += 1
    def fits(off, W, nj):
        if off % 128:
            return False
        for j in range(nj):
            lo, hi = off + j * W, off + (j + 1) * W - 1
            if lo // 512 != hi // 512:
                return False
        return True

    chunks = []
    cur, used = [], 0  # used = next free col; holes recorded per chunk
    holes = []
    for g in segs_all:
        nj = 2 if g["dr"] else 1
        w = nj * g["W"]
        off = used
        while off + w <= cap and not fits(off, g["W"], nj):
            off += 128
        if off + w > cap:
            chunks.append((cur, used, holes))
            cur, used, holes = [], 0, []
            off = 0
            while not fits(off, g["W"], nj):
                off += 128
        if off > used:
            holes.append((used, off - used))
        g["off"] = off
        cur.append(g)
        used = off + w
    if cur:
        chunks.append((cur, used, holes))
    n_av = len(segs_all)
    return chunks, n_av


def build_nc(nblk: int):
    """Build the single SPMD Bass graph (same program on all 8 cores)."""
    L = nblk * 128
    NS = (L + 511) // 512

    def fs(s):
        return min(512, L - 512 * s)

    def kbmax(s):
        return min(4 * s + (fs(s) + 127) // 128, nblk)

    nc = bacc.Bacc(
        "TRN2",
        target_bir_lowering=False,
        debug=False,
        enable_asserts=False,
        num_devices=N_CORES,
    )
    xt_d = nc.dram_tensor("xt", [4, 128, L], BF16, kind="ExternalInput").ap()
    wq_d = nc.dram_tensor("wq", [128, 1024], BF16, kind="ExternalInput").ap()
    wk_d = nc.dram_tensor("wk", [128, 1024], BF16, kind="ExternalInput").ap()
    wv_d = nc.dram_tensor("wv", [128, 1024], BF16, kind="ExternalInput").ap()
    wo_d = nc.dram_tensor("wo", [128, 1024], BF16, kind="ExternalInput").ap()
    m_d = nc.dram_tensor("m", [128, nblk], F32, kind="ExternalInput").ap()
    tri_d = nc.dram_tensor("tri", [128, 128], BF16, kind="ExternalInput").ap()
    out_d = nc.dram_tensor("out", [T, D], BF16, kind="ExternalOutput").ap()

    with tile.TileContext(nc) as tc:
        with (
            tc.tile_pool(name="const", bufs=1) as cpool,
            tc.tile_pool(name="work", bufs=3) as wpool,
            tc.tile_pool(name="ps", bufs=2, space="PSUM") as pspool,
            tc.tile_pool(name="pwork", bufs=5) as ppool,
            tc.tile_pool(name="s_ps", bufs=2, space="PSUM") as spool,
            tc.tile_pool(name="o_ps", bufs=2, space="PSUM") as opool,
        ):
            # ---- persistent inputs -> SBUF, one dma_start per tensor ----
            wq_sb = cpool.tile([128, 1024], BF16, tag="wq", name="wq_sb")
            wk_sb = cpool.tile([128, 1024], BF16, tag="wk", name="wk_sb")
            wv_sb = cpool.tile([128, 1024], BF16, tag="wv", name="wv_sb")
            wo_sb = cpool.tile([128, 1024], BF16, tag="wo", name="wo_sb")
            m_sb = cpool.tile([128, nblk], F32, tag="m", name="m_sb")
            tri_sb = cpool.tile([128, 128], BF16, tag="tri", name="tri_sb")
            xt = [
                cpool.tile([128, L], BF16, tag=f"xt{d4}", name=f"xt{d4}")
                for d4 in range(4)
            ]
            startup_engines = [nc.sync, nc.gpsimd, nc.gpsimd, nc.sync]

            def fetch_xt(s, startup=False):
                if s >= NS:
                    return
                c0, w = 512 * s, fs(s)
                for d4 in range(4):
                    eng = startup_engines[d4] if startup else nc.sync
                    eng.dma_start(
                        xt[d4][:, c0 : c0 + w],
                        xt_d[d4, :, c0 : c0 + w],
                    )

            # issue order staggers the ~2MB initial load so the bytes the
            # first qkv quanta need (wq/wk first halves, xt superblock 0)
            # land first.  scalar's queue is blocked ~1.3us by the exp
            # ACT_TABLE_LOAD, so it gets no first-wave DMA.
            nc.sync.dma_start(wq_sb[:, 0:512], wq_d[:, 0:512])
            nc.sync.dma_start(wk_sb[:, 0:512], wk_d[:, 0:512])
            fetch_xt(0, startup=True)
            nc.sync.dma_start(wq_sb[:, 512:1024], wq_d[:, 512:1024])
            nc.sync.dma_start(wk_sb[:, 512:1024], wk_d[:, 512:1024])
            nc.scalar.dma_start(wv_sb[:], wv_d[:])
            nc.gpsimd.dma_start(m_sb[:], m_d[:])
            nc.gpsimd.dma_start(tri_sb[:], tri_d[:])
            fetch_xt(1, startup=True)
            nc.scalar.dma_start(wo_sb[:], wo_d[:])

            bias_sb = cpool.tile([128, 1], F32, tag="bias", name="bias_sb")
            nc.vector.memset(bias_sb[:], EXP_BIAS)
            ones_sb = cpool.tile([1, 64], BF16, tag="ones", name="ones_sb")
            nc.vector.memset(ones_sb[:], 1.0)

            # HAM warm-up: dummy matmuls during the DMA lead-in so the PE
            # clock gate is granted before real work starts.
            wu_sb = cpool.tile([128, 128], BF16, tag="wu", name="wu_sb")
            nc.vector.memset(wu_sb[:], 0.0)
            wu_ps = pspool.tile([128, 512], F32, tag="ps", name="wu_ps")
            for _ in range(112):
                nc.tensor.matmul(
                    wu_ps[:, :128], lhsT=wu_sb[:], rhs=wu_sb[:],
                    start=True, stop=True,
                )

            # ---- qkv projection quanta ----
            qt = {}
            kt = {}
            vp = {}  # pair index -> (128, 520) fp8 tile; or single (128,260)
            built_pairs = set()

            def qk_quantum(s, hp, which, wsb, store):
                def run():
                    w = fs(s)
                    ps = pspool.tile([128, 512], F32, tag="ps", name="ps")
                    for d4 in range(4):
                        col = 128 * (4 * hp + d4)
                        nc.tensor.matmul(
                            ps[:, :w],
                            lhsT=wsb[:, col : col + 128],
                            rhs=xt[d4][:, 512 * s : 512 * s + w],
                            start=(d4 == 0),
                            stop=(d4 == 3),
                        )
                    dst = cpool.tile(
                        [128, w], BF16, tag=f"{which}t{hp}_{s}",
                        name=f"{which}t{hp}_{s}",
                    )
                    nc.vector.tensor_copy(dst[:], ps[:, :w])
                    store[(hp, s)] = dst

                return run

            def v_quantum(pi, single):
                """Build V pair tile pi (key blocks 2pi, 2pi+1), fp8."""
                def run():
                    njs = 1 if single else 2
                    vt = cpool.tile(
                        [128, 4 * 65 * njs], BF16, tag=f"v{pi}", name=f"v{pi}"
                    )
                    v4 = vt[:].rearrange("p (h j c) -> p h j c", j=njs, c=65)
                    for j in range(njs):
                        kb = 2 * pi + j
                        ps = pspool.tile([128, 512], F32, tag="ps", name="ps")
                        for d4 in range(4):
                            nc.tensor.matmul(
                                ps[:, :256],
                                lhsT=xt[d4][:, 128 * kb : 128 * (kb + 1)],
                                rhs=wv_sb[:, 256 * d4 : 256 * (d4 + 1)],
                                start=(d4 == 0),
                                stop=(d4 == 3),
                            )
                        nc.vector.tensor_copy(
                            v4[:, :, j, 0:64],
                            ps[:, :256].rearrange("p (h c) -> p h c", c=64),
                        )
                    nc.gpsimd.memset(
                        vt[:].rearrange("p (g c) -> p g c", c=65)[:, :, 64:65],
                        1.0,
                    )
                    vp[pi] = (vt, njs)

                return run

            def qkv_quanta(s):
                """Quanta list: xt prefetch, q/k projections, new V pairs."""
                quanta = []
                if s + 2 < NS:
                    quanta.append(lambda s=s: fetch_xt(s + 2))
                for hp in range(2):
                    quanta.append(qk_quantum(s, hp, "q", wq_sb, qt))
                    quanta.append(qk_quantum(s, hp, "k", wk_sb, kt))
                KB = kbmax(s)
                pi = 0
                while 2 * pi < KB:
                    if pi not in built_pairs:
                        built_pairs.add(pi)
                        single = 2 * pi + 1 >= nblk
                        quanta.append(v_quantum(pi, single))
                    pi += 1
                return quanta

            filler = deque()

            def emit_fill(n):
                for _ in range(min(n, len(filler))):
                    filler.popleft()()

            # ---- attention ----
            def attention_pair(s, hp, o_ps2, ot_sb, early_cbs):
                F = fs(s)
                cap = 512 if F <= 128 else 1024
                chunks, _ = plan_segs(s, nblk, cap)
                n_av = sum(
                    len(g["members"]) for segs, _, _ in chunks for g in segs
                )
                done = [0, 0]

                def do_avs(hi, segs, p8t):
                    h = 2 * hp + hi
                    for g in segs:
                        W, off, qoff = g["W"], g["off"], g["qoff"]
                        for j, (kb, c0, pad) in enumerate(g["members"]):
                            pi, jj = divmod(kb, 2)
                            vt, njs = vp[pi]
                            v4 = vt[:].rearrange(
                                "p (hh j c) -> p hh j c", j=njs, c=65
                            )
                            nc.tensor.matmul(
                                o_ps2[hi][0:65, qoff : qoff + W],
                                lhsT=v4[:, h, jj, :],
                                rhs=p8t[:, off + j * W : off + (j + 1) * W],
                                start=(done[hi] == 0),
                                stop=(done[hi] == n_av - 1),
                            )
                            done[hi] += 1

                prev = None
                for ci, (segs, used, holes) in enumerate(chunks):
                    ps2 = []
                    for hi in range(2):
                        p0 = 64 * hi
                        s_ps = spool.tile([128, 1024], F32, tag="s", name="s_ps")
                        for hoff, hw in holes:
                            nc.vector.memset(s_ps[:, hoff : hoff + hw], 0.0)
                        for g in segs:
                            W, off, qoff = g["W"], g["off"], g["qoff"]
                            for j, (kb, c0, pad) in enumerate(g["members"]):
                                tck, o4 = divmod(kb, 4)
                                nc.tensor.matmul(
                                    s_ps[:, off + j * W : off + (j + 1) * W],
                                    lhsT=kt[(hp, tck)][
                                        p0 : p0 + 64,
                                        128 * o4 : 128 * o4 + 128,
                                    ],
                                    rhs=qt[(hp, s)][
                                        p0 : p0 + 64, qoff : qoff + W
                                    ],
                                    start=True,
                                    stop=True,
                                )
                        ps2.append(s_ps)
                    pb2 = []
                    for hi in range(2):
                        p8t = ppool.tile([128, 1024], BF16, tag="p", name="pt")
                        nc.scalar.activation(
                            p8t[:, :used], ps2[hi][:, :used], AF.Exp,
                            bias=bias_sb[:],
                        )
                        for g in segs:
                            W, off = g["W"], g["off"]
                            for j, (kb, c0, pad) in enumerate(g["members"]):
                                if c0 is not None:
                                    nc.gpsimd.tensor_mul(
                                        p8t[:, off + j * W + c0 :
                                            off + j * W + c0 + 128],
                                        p8t[:, off + j * W + c0 :
                                            off + j * W + c0 + 128],
                                        tri_sb[:],
                                    )
                                if pad:
                                    nc.gpsimd.memset(
                                        p8t[:, off + j * W : off + j * W + pad],
                                        0.0,
                                    )
                        pb2.append(p8t)
                    if ci < len(early_cbs) and early_cbs[ci] is not None:
                        early_cbs[ci]()
                    if prev is not None:
                        for hi in range(2):
                            do_avs(hi, prev[0], prev[1][hi])
                    emit_fill(1)
                    prev = (segs, pb2)
                for ci in range(len(chunks), len(early_cbs)):
                    if early_cbs[ci] is not None:
                        early_cbs[ci]()
                # PE work to chew on while the last chunk's exp/tri land
                emit_fill(2)
                for hi in range(2):
                    do_avs(hi, prev[0], prev[1][hi])

            # ---- normalize: drain O, build 1/l, defer bcast+mul ----
            def start_normalize(s, hp, o_ps2, ot_sb):
                F = fs(s)
                nq = (F + 127) // 128
                lcols = []
                lrows = []
                dsts = []
                for hi in range(2):
                    p0 = 64 * hi
                    dst = ot_sb[p0 : p0 + 64, 0:F]
                    if hi == 0:
                        # partitions match (0-63): DVE copy
                        nc.vector.tensor_copy(dst, o_ps2[hi][0:64, :F])
                    else:
                        # partition shift 0-63 -> 64-127: only ACT can
                        nc.scalar.activation(dst, o_ps2[hi][0:64, :F], AF.Copy)
                    lrow = wpool.tile([1, 512], F32, tag=f"lr{hi}", name="lrow")
                    nc.vector.tensor_copy(lrow[0:1, :F], o_ps2[hi][64:65, :F])
                    lrows.append(lrow)
                    if nq > 1:
                        lcol = wpool.tile(
                            [128, 4], F32, tag=f"lc{hi}", name="lcol"
                        )
                        nc.sync.dma_start(
                            lcol[:, 0:nq],
                            lrow[0:1, :F].rearrange("o (p c) -> o p c", c=nq),
                        )
                        lcols.append(lcol)
                    dsts.append(dst)
                rcs = [None, None]

                def fin_a():
                    for hi in range(2):
                        rc = wpool.tile([1, 512], BF16, tag=f"rr{hi}", name="rc")
                        with nc.allow_low_precision(
                            reason="1/l in bf16: 0.4% rms, well under gate"
                        ):
                            if nq == 1:
                                # end-of-kernel critical chain: direct
                                # reciprocal skips two DMA round trips
                                nc.vector.reciprocal(
                                    rc[0:1, :F], lrows[hi][0:1, :F]
                                )
                                rcs[hi] = rc
                                continue
                            rcol = wpool.tile(
                                [128, 4], BF16, tag=f"rc{hi}", name="rcol"
                            )
                            nc.vector.reciprocal(
                                rcol[:, 0:nq], lcols[hi][:, 0:nq]
                            )
                        nc.sync.dma_start(
                            rc[0:1, :F].rearrange("o (p c) -> o p c", c=nq),
                            rcol[:, 0:nq],
                        )
                        rcs[hi] = rc

                def norm_muls():
                    # 1/l row -> 64 partitions via a bf16 PE outer product
                    # (runs as an outproj-quantum filler, long after rc is
                    # ready, so the PE never waits on the reciprocal chain)
                    bc_ps = pspool.tile([128, 512], F32, tag="ps", name="bc_ps")
                    for hi in range(2):
                        nc.tensor.matmul(
                            bc_ps[64 * hi : 64 * hi + 64, :F],
                            lhsT=ones_sb[0:1, :],
                            rhs=rcs[hi][0:1, :F],
                            start=True,
                            stop=True,
                        )
                    for hi in range(2):
                        nc.vector.tensor_mul(
                            dsts[hi], dsts[hi],
                            bc_ps[64 * hi : 64 * hi + 64, :F],
                        )

                return fin_a, norm_muls

            # ---- out-projection quanta (prefixed by deferred normalize) ----
            def outproj_quanta(s, ot_sbs, muls):
                F = fs(s)
                quanta = list(muls)

                def qblock(qi):
                    def run():
                        y_ps = pspool.tile([128, 512], F32, tag="ps", name="ps")
                        for hp in range(2):
                            nc.tensor.matmul(
                                y_ps[:],
                                lhsT=ot_sbs[hp][:, 128 * qi : 128 * (qi + 1)],
                                rhs=wo_sb[:, 512 * hp : 512 * (hp + 1)],
                                start=(hp == 0),
                                stop=(hp == 1),
                            )
                        ob = wpool.tile([128, 512], BF16, tag="ob", name="ob")
                        qg = 4 * s + qi
                        nc.vector.tensor_scalar_mul(
                            ob[:], y_ps[:], m_sb[:, qg : qg + 1]
                        )
                        row = 512 * s + 128 * qi
                        nc.sync.dma_start(out_d[row : row + 128, :], ob[:])

                    return run

                for qi in range((F + 127) // 128):
                    quanta.append(qblock(qi))
                return quanta

            # ---- main loop ----
            for q in qkv_quanta(0):
                q()
            if NS > 1:
                filler.extend(qkv_quanta(1))

            pending = None  # fin_a of previous pair
            pending_mulss = None  # norm_muls pair for superblock s-1
            pending_out = None  # (s-1, ot_sbs) awaiting its hp=1 fin_a
            for s in range(NS):
                ot_sbs = [
                    wpool.tile([128, 512], BF16, tag=f"ot{hp}", name=f"ot{hp}")
                    for hp in range(2)
                ]
                mulss = []
                for hp in range(2):
                    o_ps2 = [
                        opool.tile([65, 512], F32, tag="o", name="o_ps")
                        for _ in range(2)
                    ]
                    prev_pending = pending
                    prev_out = pending_out if hp == 0 else None
                    prev_mulss = pending_mulss

                    def early0(pp=prev_pending):
                        if pp is not None:
                            pp()

                    def early1(po=prev_out, pm=prev_mulss):
                        if po is not None:
                            ps_, ots_ = po
                            filler.extend(outproj_quanta(ps_, ots_, pm))

                    attention_pair(s, hp, o_ps2, ot_sbs[hp], [early0, early1])
                    if hp == 0 and prev_out is not None:
                        pending_out = None
                    fa, muls = start_normalize(s, hp, o_ps2, ot_sbs[hp])
                    pending = fa
                    mulss.append(muls)
                # flush remaining fillers so qt/kt(s+1) exist before use,
                # THEN queue qkv(s+2) for weaving into superblock s+1
                emit_fill(len(filler))
                if s + 2 < NS:
                    filler.extend(qkv_quanta(s + 2))
                pending_out = (s, ot_sbs)
                pending_mulss = mulss
            # tail: last pair's normalize + last superblock's outproj
            # (hp0's normalize-mul already ran inside the last pair)
            pending()
            for q in outproj_quanta(
                pending_out[0], pending_out[1], pending_mulss
            ):
                q()

    nc.compile()
    return nc


def make_in_maps(x, m, w_qkv, w_out, nblk: int):
    """Host-side sharding/packing: core c = (batch c//2, head-group c%2)."""
    L = nblk * 128
    tri = np.where(
        np.arange(128)[None, :] >= np.arange(128)[:, None], 1.0, 0.0
    ).astype(NP_BF16)
    in_maps = []
    for c in range(N_CORES):
        b, g = divmod(c, 2)
        xt = np.ascontiguousarray(x[b].T[:, :L]).astype(NP_BF16).reshape(
            4, 128, L
        )
        wq = np.empty((128, 1024), np.float32)
        wk = np.empty((128, 1024), np.float32)
        for hp in range(2):
            for d4 in range(4):
                rows = slice(128 * d4, 128 * (d4 + 1))
                qcol = 256 * g + 128 * hp
                col = 128 * (4 * hp + d4)
                wq[:, col : col + 128] = w_qkv[rows, qcol : qcol + 128] * SCALE
                wk[:, col : col + 128] = w_qkv[rows, 512 + qcol : 512 + qcol + 128]
        wv = np.empty((128, 1024), np.float32)
        for d4 in range(4):
            wv[:, 256 * d4 : 256 * (d4 + 1)] = w_qkv[
                128 * d4 : 128 * (d4 + 1), 1024 + 256 * g : 1024 + 256 * (g + 1)
            ]
        wo = np.empty((128, 1024), np.float32)
        for hp in range(2):
            r0 = 256 * g + 128 * hp
            wo[:, 512 * hp : 512 * (hp + 1)] = w_out[r0 : r0 + 128, :]
        mp = np.ascontiguousarray(
            m[b, :L, 0].reshape(nblk, 128).T
        ).astype(np.float32)
        in_maps.append(
            {
                "xt": xt,
                "wq": wq.astype(NP_BF16),
                "wk": wk.astype(NP_BF16),
                "wv": wv.astype(NP_BF16),
                "wo": wo.astype(NP_BF16),
                "m": mp,
                "tri": tri,
            }
        )
    return in_maps


def postprocess(results, x, m, b_out):
    out = np.zeros((B, T, D), np.float32)
    for b in range(B):
        out[b] = results[2 * b]["out"].astype(np.float32) + results[
            2 * b + 1
        ]["out"].astype(np.float32)
    out += b_out[None, None, :].astype(np.float32) * m.astype(np.float32)
    return out


def kernel(x, m, w_qkv, w_out, b_out):
    lengths = m[:, :, 0].astype(np.int64).sum(axis=1)
    nblk = max(1, int(-(-lengths.max() // 128)))
    nc = build_nc(nblk)
    in_maps = make_in_maps(x, m, w_qkv, w_out, nblk)
    res = bass_utils.run_bass_kernel_spmd(nc, in_maps, core_ids=list(range(N_CORES)))
    return postprocess(res.results, x, m, b_out)


# revision 25
# speedup vs baseline: 1.0088x; 1.0088x over previous
"""Trainium2 Bass kernel for masked causal multi-head attention.

Problem (hardcoded):
    x: (4, 2048, 512) f32, m: (4, 2048, 1) f32 (prefix 0/1 mask),
    w_qkv: (512, 1536) f32, w_out: (512, 512) f32, b_out: (512,) f32
    out = (softmax(mask(QK^T/8)) V) @ w_out + b_out, masked by m.

Sharding: 8 cores = 4 batches x 2 head-groups (4 heads each).  Each core
computes the qkv projection for its (batch, head-group), flash-style causal
attention, and a partial out-projection; the host sums the two partials
per batch and adds b_out.

Kernel structure (all compute bf16, accumulation f32 in PSUM; fp8 was
tried for P/V and rejected: quantization error ~2.5e-2 vs the 2e-2 gate,
because the output magnitude scales as 1/sqrt(N_eff) exactly like the
quantization noise, so there is no averaging benefit):
  - Q^T, K^T in (dh, t) layout, two heads stacked per 128 partitions ->
    scores computed transposed: S^T (k, q), so softmax needs no transposes.
    No max-subtraction: scores are ~N(0,1), |s| <= ~7; exp(s-4) is safe.
  - exp on the scalar engine (its ONLY job: exp column count matches S
    column count, so the scalar engine is the second-busiest after PE).
    Causal triangle is a post-exp multiply by a 0/1 triangle (gpsimd).
  - V in key-block-pair tiles with an extra all-ones 65th column per head
    (row-sum trick); O^T = V_aug^T P accumulated over key blocks in PSUM.
    Diagonal-pair second members are computed padded to the pair width;
    the invalid P region is zeroed by a gpsimd memset before AV.
  - 1/l is broadcast across partitions with gpsimd partition_broadcast
    (no PE involvement, unlike the old ones(1,64) matmul broadcast which
    cost 22us of PE); the per-head normalize multiplies are deferred
    into the out-projection quanta (deep slack, off every critical path).
  - qkv projection for superblock s+1 and out-projection for s-1 are cut
    into small quanta and woven between attention chunks so the PE stream
    never goes dry (the HAM clock gate halves the PE clock for ~10us after
    any idle window -- gaplessness is worth more than instruction count).
  - Engine assignment: Scalar=exp only; DVE=all PSUM readers (q/k/v
    drains, O drains, l-row extracts, reciprocals, outproj mask-scales);
    Pool=SBUF elementwise (triangle, normalize muls, pad/ones memsets,
    l-transpose DMA issues); Sync=bulk DMA issue.
  - All host-side tensors are packed so every weight is ONE contiguous
    dma_start (DGE config on an engine sequencer costs ~600ns; the old
    per-block DMAs serialized ~15us of config at startup).
"""

import sys
from collections import deque

import numpy as np

try:
    import concourse.bass as bass  # noqa: F401
except ImportError:  # pragma: no cover
    sys.path.insert(0, "/opt/trn_rl_repo")

import concourse.bacc as bacc
import concourse.mybir as mybir
import concourse.tile as tile
from concourse import bass_utils

F32 = mybir.dt.float32
BF16 = mybir.dt.bfloat16
F8 = mybir.dt.float8e4
NP_BF16 = mybir.dt.np(BF16)
AF = mybir.ActivationFunctionType
DR = mybir.MatmulPerfMode.DoubleRow

B, T, D, H = 4, 2048, 512, 8
DH = D // H  # 64
G = 2  # head groups (cores per batch)
SCALE = DH**-0.5
EXP_BIAS = -4.0  # exp(s-4): keeps P in fp8e4m3 range; cancels in softmax
N_CORES = 8


def plan_segs(s, nblk, cap=1024):
    """Segment plan for superblock s: DoubleRow pairs + trailing single.

    Returns (chunks, n_av) where chunks is a list of (segs, used) and each
    seg is a dict: off (col offset in the 1024-wide chunk tile), W (member
    width), qoff (query offset of the segment in the superblock), dr
    (DoubleRow pair or single), members [(kb, c0_tri or None, pad)].
    """
    L = nblk * 128
    F = min(512, L - 512 * s)
    KB = min(4 * s + (F + 127) // 128, nblk)
    segs_all = []
    kb = 0
    while kb < KB:
        qa = max(0, 128 * (kb - 4 * s))
        W = F - qa
        if kb + 1 < KB:
            members = []
            for j, kbx in enumerate((kb, kb + 1)):
                dq = 128 * (kbx - 4 * s)  # diag col within segment (abs)
                c0 = dq - qa if dq >= qa else None  # tri pos, None=off-diag
                pad = (dq - qa) if (j == 1 and dq > qa) else 0
                members.append((kbx, c0, pad))
            segs_all.append(dict(W=W, qoff=qa, dr=True, members=members))
            kb += 2
        else:
            dq = 128 * (kb - 4 * s)
            c0 = dq - qa if dq >= qa else None
            segs_all.append(
                dict(W=W, qoff=qa, dr=False, members=[(kb, c0, 0)])
            )
            kb += 1
    def fits(off, W, nj):
        if off % 128:
            return False
        for j in range(nj):
            lo, hi = off + j * W, off + (j + 1) * W - 1
            if lo // 512 != hi // 512:
                return False
        return True

    chunks = []
    cur, used = [], 0  # used = next free col; holes recorded per chunk
    holes = []
    for g in segs_all:
        nj = 2 if g["dr"] else 1
        w = nj * g["W"]
        off = used
        while off + w <= cap and not fits(off, g["W"], nj):
            off += 128
        if off + w > cap:
            chunks.append((cur, used, holes))
            cur, used, holes = [], 0, []
            off = 0
            while not fits(off, g["W"], nj):
                off += 128
        if off > used:
            holes.append((used, off - used))
        g["off"] = off
        cur.append(g)
        used = off + w
    if cur:
        chunks.append((cur, used, holes))
    n_av = len(segs_all)
    return chunks, n_av


def build_nc(nblk: int):
    """Build the single SPMD Bass graph (same program on all 8 cores)."""
    L = nblk * 128
    NS = (L + 511) // 512

    def fs(s):
        return min(512, L - 512 * s)

    def kbmax(s):
        return min(4 * s + (fs(s) + 127) // 128, nblk)

    nc = bacc.Bacc(
        "TRN2",
        target_bir_lowering=False,
        debug=False,
        enable_asserts=False,
        num_devices=N_CORES,
    )
    xt_d = nc.dram_tensor("xt", [4, 128, L], BF16, kind="ExternalInput").ap()
    wq_d = nc.dram_tensor("wq", [128, 1024], BF16, kind="ExternalInput").ap()
    wk_d = nc.dram_tensor("wk", [128, 1024], BF16, kind="ExternalInput").ap()
    wv_d = nc.dram_tensor("wv", [128, 1024], BF16, kind="ExternalInput").ap()
    wo_d = nc.dram_tensor("wo", [128, 1024], BF16, kind="ExternalInput").ap()
    m_d = nc.dram_tensor("m", [128, nblk], F32, kind="ExternalInput").ap()
    tri_d = nc.dram_tensor("tri", [128, 128], BF16, kind="ExternalInput").ap()
    out_d = nc.dram_tensor("out", [T, D], BF16, kind="ExternalOutput").ap()

    with tile.TileContext(nc) as tc:
        with (
            tc.tile_pool(name="const", bufs=1) as cpool,
            tc.tile_pool(name="work", bufs=3) as wpool,
            tc.tile_pool(name="ps", bufs=2, space="PSUM") as pspool,
            tc.tile_pool(name="pwork", bufs=5) as ppool,
            tc.tile_pool(name="s_ps", bufs=2, space="PSUM") as spool,
            tc.tile_pool(name="o_ps", bufs=2, space="PSUM") as opool,
        ):
            # ---- persistent inputs -> SBUF, one dma_start per tensor ----
            wq_sb = cpool.tile([128, 1024], BF16, tag="wq", name="wq_sb")
            wk_sb = cpool.tile([128, 1024], BF16, tag="wk", name="wk_sb")
            wv_sb = cpool.tile([128, 1024], BF16, tag="wv", name="wv_sb")
            wo_sb = cpool.tile([128, 1024], BF16, tag="wo", name="wo_sb")
            m_sb = cpool.tile([128, nblk], F32, tag="m", name="m_sb")
            tri_sb = cpool.tile([128, 128], BF16, tag="tri", name="tri_sb")
            xt = [
                cpool.tile([128, L], BF16, tag=f"xt{d4}", name=f"xt{d4}")
                for d4 in range(4)
            ]
            startup_engines = [nc.sync, nc.gpsimd, nc.gpsimd, nc.sync]

            def fetch_xt(s, startup=False):
                if s >= NS:
                    return
                c0, w = 512 * s, fs(s)
                for d4 in range(4):
                    eng = startup_engines[d4] if startup else nc.sync
                    eng.dma_start(
                        xt[d4][:, c0 : c0 + w],
                        xt_d[d4, :, c0 : c0 + w],
                    )

            # issue order staggers the ~2MB initial load so the bytes the
            # first qkv quanta need (wq/wk first halves, xt superblock 0)
            # land first.  scalar's queue is blocked ~1.3us by the exp
            # ACT_TABLE_LOAD, so it gets no first-wave DMA.
            nc.sync.dma_start(wq_sb[:, 0:512], wq_d[:, 0:512])
            nc.sync.dma_start(wk_sb[:, 0:512], wk_d[:, 0:512])
            fetch_xt(0, startup=True)
            nc.sync.dma_start(wq_sb[:, 512:1024], wq_d[:, 512:1024])
            nc.sync.dma_start(wk_sb[:, 512:1024], wk_d[:, 512:1024])
            nc.scalar.dma_start(wv_sb[:], wv_d[:])
            nc.gpsimd.dma_start(m_sb[:], m_d[:])
            nc.gpsimd.dma_start(tri_sb[:], tri_d[:])
            fetch_xt(1, startup=True)
            nc.scalar.dma_start(wo_sb[:], wo_d[:])

            bias_sb = cpool.tile([128, 1], F32, tag="bias", name="bias_sb")
            nc.vector.memset(bias_sb[:], EXP_BIAS)
            ones_sb = cpool.tile([1, 64], BF16, tag="ones", name="ones_sb")
            nc.vector.memset(ones_sb[:], 1.0)

            # HAM warm-up: dummy matmuls during the DMA lead-in so the PE
            # clock gate is granted before real work starts.
            wu_sb = cpool.tile([128, 128], BF16, tag="wu", name="wu_sb")
            nc.vector.memset(wu_sb[:], 0.0)
            wu_ps = pspool.tile([128, 512], F32, tag="ps", name="wu_ps")
            for _ in range(112):
                nc.tensor.matmul(
                    wu_ps[:, :128], lhsT=wu_sb[:], rhs=wu_sb[:],
                    start=True, stop=True,
                )

            # ---- qkv projection quanta ----
            qt = {}
            kt = {}
            vp = {}  # pair index -> (128, 520) fp8 tile; or single (128,260)
            built_pairs = set()

            def qk_quantum(s, hp, which, wsb, store):
                def run():
                    w = fs(s)
                    ps = pspool.tile([128, 512], F32, tag="ps", name="ps")
                    for d4 in range(4):
                        col = 128 * (4 * hp + d4)
                        nc.tensor.matmul(
                            ps[:, :w],
                            lhsT=wsb[:, col : col + 128],
                            rhs=xt[d4][:, 512 * s : 512 * s + w],
                            start=(d4 == 0),
                            stop=(d4 == 3),
                        )
                    dst = cpool.tile(
                        [128, w], BF16, tag=f"{which}t{hp}_{s}",
                        name=f"{which}t{hp}_{s}",
                    )
                    nc.vector.tensor_copy(dst[:], ps[:, :w])
                    store[(hp, s)] = dst

                return run

            def v_quantum(pi, single):
                """Build V pair tile pi (key blocks 2pi, 2pi+1), fp8."""
                def run():
                    njs = 1 if single else 2
                    vt = cpool.tile(
                        [128, 4 * 65 * njs], BF16, tag=f"v{pi}", name=f"v{pi}"
                    )
                    v4 = vt[:].rearrange("p (h j c) -> p h j c", j=njs, c=65)
                    for j in range(njs):
                        kb = 2 * pi + j
                        ps = pspool.tile([128, 512], F32, tag="ps", name="ps")
                        for d4 in range(4):
                            nc.tensor.matmul(
                                ps[:, :256],
                                lhsT=xt[d4][:, 128 * kb : 128 * (kb + 1)],
                                rhs=wv_sb[:, 256 * d4 : 256 * (d4 + 1)],
                                start=(d4 == 0),
                                stop=(d4 == 3),
                            )
                        nc.vector.tensor_copy(
                            v4[:, :, j, 0:64],
                            ps[:, :256].rearrange("p (h c) -> p h c", c=64),
                        )
                    nc.gpsimd.memset(
                        vt[:].rearrange("p (g c) -> p g c", c=65)[:, :, 64:65],
                        1.0,
                    )
                    vp[pi] = (vt, njs)

                return run

            def qkv_quanta(s):
                """Quanta list: xt prefetch, q/k projections, new V pairs."""
                quanta = []
                if s + 2 < NS:
                    quanta.append(lambda s=s: fetch_xt(s + 2))
                for hp in range(2):
                    quanta.append(qk_quantum(s, hp, "q", wq_sb, qt))
                    quanta.append(qk_quantum(s, hp, "k", wk_sb, kt))
                KB = kbmax(s)
                pi = 0
                while 2 * pi < KB:
                    if pi not in built_pairs:
                        built_pairs.add(pi)
                        single = 2 * pi + 1 >= nblk
                        quanta.append(v_quantum(pi, single))
                    pi += 1
                return quanta

            filler = deque()

            def emit_fill(n):
                for _ in range(min(n, len(filler))):
                    filler.popleft()()

            # ---- attention: one flat chunk-event stream across ALL pairs
            # with cross-pair software pipelining -- the next pair's first
            # S-chunk is always emitted before this pair's final AVs, so
            # the PE never waits on exp/tri at a pair boundary ----
            def emit_s_chunk(pr, segs, used, holes):
                s, hp = pr["s"], pr["hp"]
                ps2 = []
                for hi in range(2):
                    p0 = 64 * hi
                    s_ps = spool.tile([128, 1024], F32, tag="s", name="s_ps")
                    for hoff, hw in holes:
                        nc.vector.memset(s_ps[:, hoff : hoff + hw], 0.0)
                    for g in segs:
                        W, off, qoff = g["W"], g["off"], g["qoff"]
                        for j, (kb, c0, pad) in enumerate(g["members"]):
                            tck, o4 = divmod(kb, 4)
                            nc.tensor.matmul(
                                s_ps[:, off + j * W : off + (j + 1) * W],
                                lhsT=kt[(hp, tck)][
                                    p0 : p0 + 64, 128 * o4 : 128 * o4 + 128
                                ],
                                rhs=qt[(hp, s)][p0 : p0 + 64, qoff : qoff + W],
                                start=True,
                                stop=True,
                            )
                    ps2.append(s_ps)
                pb2 = []
                for hi in range(2):
                    p8t = ppool.tile([128, 1024], BF16, tag="p", name="pt")
                    nc.scalar.activation(
                        p8t[:, :used], ps2[hi][:, :used], AF.Exp,
                        bias=bias_sb[:],
                    )
                    for g in segs:
                        W, off = g["W"], g["off"]
                        for j, (kb, c0, pad) in enumerate(g["members"]):
                            if c0 is not None:
                                nc.gpsimd.tensor_mul(
                                    p8t[:, off + j * W + c0 :
                                        off + j * W + c0 + 128],
                                    p8t[:, off + j * W + c0 :
                                        off + j * W + c0 + 128],
                                    tri_sb[:],
                                )
                            if pad:
                                nc.gpsimd.memset(
                                    p8t[:, off + j * W : off + j * W + pad],
                                    0.0,
                                )
                    pb2.append(p8t)
                return pb2

            def emit_avs(pr, segs, pb2):
                for hi in range(2):
                    h = 2 * pr["hp"] + hi
                    for g in segs:
                        W, off, qoff = g["W"], g["off"], g["qoff"]
                        for j, (kb, c0, pad) in enumerate(g["members"]):
                            pi, jj = divmod(kb, 2)
                            vt, njs = vp[pi]
                            v4 = vt[:].rearrange(
                                "p (hh j c) -> p hh j c", j=njs, c=65
                            )
                            nc.tensor.matmul(
                                pr["o_ps2"][hi][0:65, qoff : qoff + W],
                                lhsT=v4[:, h, jj, :],
                                rhs=pb2[hi][:, off + j * W : off + (j + 1) * W],
                                start=(pr["done"][hi] == 0),
                                stop=(pr["done"][hi] == pr["n_av"] - 1),
                            )
                            pr["done"][hi] += 1

            # ---- normalize: drain O, build 1/l, defer bcast+mul ----
            def start_normalize(s, hp, o_ps2, ot_sb):
                F = fs(s)
                nq = (F + 127) // 128
                lcols = []
                lrows = []
                dsts = []
                for hi in range(2):
                    p0 = 64 * hi
                    dst = ot_sb[p0 : p0 + 64, 0:F]
                    if hi == 0:
                        # partitions match (0-63): DVE copy
                        nc.vector.tensor_copy(dst, o_ps2[hi][0:64, :F])
                    else:
                        # partition shift 0-63 -> 64-127: only ACT can
                        nc.scalar.activation(dst, o_ps2[hi][0:64, :F], AF.Copy)
                    lrow = wpool.tile([1, 512], F32, tag=f"lr{hi}", name="lrow")
                    nc.vector.tensor_copy(lrow[0:1, :F], o_ps2[hi][64:65, :F])
                    lrows.append(lrow)
                    if nq > 1:
                        lcol = wpool.tile(
                            [128, 4], F32, tag=f"lc{hi}", name="lcol"
                        )
                        nc.sync.dma_start(
                            lcol[:, 0:nq],
                            lrow[0:1, :F].rearrange("o (p c) -> o p c", c=nq),
                        )
                        lcols.append(lcol)
                    dsts.append(dst)
                rcs = [None, None]

                def fin_a():
                    for hi in range(2):
                        rc = wpool.tile([1, 512], BF16, tag=f"rr{hi}", name="rc")
                        with nc.allow_low_precision(
                            reason="1/l in bf16: 0.4% rms, well under gate"
                        ):
                            if nq == 1:
                                # end-of-kernel critical chain: direct
                                # reciprocal skips two DMA round trips
                                nc.vector.reciprocal(
                                    rc[0:1, :F], lrows[hi][0:1, :F]
                                )
                                rcs[hi] = rc
                                continue
                            rcol = wpool.tile(
                                [128, 4], BF16, tag=f"rc{hi}", name="rcol"
                            )
                            nc.vector.reciprocal(
                                rcol[:, 0:nq], lcols[hi][:, 0:nq]
                            )
                        nc.sync.dma_start(
                            rc[0:1, :F].rearrange("o (p c) -> o p c", c=nq),
                            rcol[:, 0:nq],
                        )
                        rcs[hi] = rc

                def norm_muls():
                    # 1/l row -> 64 partitions via a bf16 PE outer product
                    # (runs as an outproj-quantum filler, long after rc is
                    # ready, so the PE never waits on the reciprocal chain)
                    bc_ps = pspool.tile([128, 512], F32, tag="ps", name="bc_ps")
                    for hi in range(2):
                        nc.tensor.matmul(
                            bc_ps[64 * hi : 64 * hi + 64, :F],
                            lhsT=ones_sb[0:1, :],
                            rhs=rcs[hi][0:1, :F],
                            start=True,
                            stop=True,
                        )
                    for hi in range(2):
                        nc.vector.tensor_mul(
                            dsts[hi], dsts[hi],
                            bc_ps[64 * hi : 64 * hi + 64, :F],
                        )

                return fin_a, norm_muls

            # ---- out-projection quanta (prefixed by deferred normalize) ----
            def outproj_quanta(s, ot_sbs, muls):
                F = fs(s)
                quanta = list(muls)

                def qblock(qi):
                    def run():
                        y_ps = pspool.tile([128, 512], F32, tag="ps", name="ps")
                        for hp in range(2):
                            nc.tensor.matmul(
                                y_ps[:],
                                lhsT=ot_sbs[hp][:, 128 * qi : 128 * (qi + 1)],
                                rhs=wo_sb[:, 512 * hp : 512 * (hp + 1)],
                                start=(hp == 0),
                                stop=(hp == 1),
                            )
                        ob = wpool.tile([128, 512], BF16, tag="ob", name="ob")
                        qg = 4 * s + qi
                        nc.vector.tensor_scalar_mul(
                            ob[:], y_ps[:], m_sb[:, qg : qg + 1]
                        )
                        row = 512 * s + 128 * qi
                        nc.sync.dma_start(out_d[row : row + 128, :], ob[:])

                    return run

                for qi in range((F + 127) // 128):
                    quanta.append(qblock(qi))
                return quanta

            # ---- main loop: flat event stream ----
            for q in qkv_quanta(0):
                q()

            pending_fins = deque()
            state = dict(pending_out=None)
            mulss_by_s = {}
            ot_sbs_by_s = {}
            prev_ev = None  # (pair-state, segs, pb2, was_last_chunk)

            def pair_final(pr):
                fa, muls = start_normalize(
                    pr["s"], pr["hp"], pr["o_ps2"], pr["ot"]
                )
                pending_fins.append(fa)
                mulss_by_s.setdefault(pr["s"], []).append(muls)
                if pr["hp"] == 1:
                    state["pending_out"] = pr["s"]

            def consume_prev():
                nonlocal prev_ev
                if prev_ev is not None:
                    p_pr, p_segs, p_pb2, p_last = prev_ev
                    emit_avs(p_pr, p_segs, p_pb2)
                    if p_last:
                        pair_final(p_pr)
                    prev_ev = None

            def mid_pair_cbs(hp):
                while pending_fins:
                    pending_fins.popleft()()
                if hp == 0 and state["pending_out"] is not None:
                    so = state["pending_out"]
                    state["pending_out"] = None
                    filler.extend(
                        outproj_quanta(so, ot_sbs_by_s[so], mulss_by_s[so])
                    )

            for s in range(NS):
                for hp in range(2):
                    if hp == 0:
                        # flush so qt/kt/v(s) exist, then queue qkv(s+1)
                        emit_fill(len(filler))
                        if s + 1 < NS:
                            filler.extend(qkv_quanta(s + 1))
                        ot_sbs_by_s[s] = [
                            wpool.tile(
                                [128, 512], BF16, tag=f"ot{h2}", name=f"ot{h2}"
                            )
                            for h2 in range(2)
                        ]
                    F = fs(s)
                    cap = 512 if F <= 128 else 1024
                    chunks, _ = plan_segs(s, nblk, cap)
                    pr = dict(
                        s=s,
                        hp=hp,
                        ot=ot_sbs_by_s[s][hp],
                        o_ps2=[
                            opool.tile([65, 512], F32, tag="o", name="o_ps")
                            for _ in range(2)
                        ],
                        n_av=sum(
                            len(g["members"])
                            for segs, _, _ in chunks
                            for g in segs
                        ),
                        done=[0, 0],
                    )
                    for ci, (segs, used, holes) in enumerate(chunks):
                        pb2 = emit_s_chunk(pr, segs, used, holes)
                        consume_prev()
                        if ci == 1 or (ci == 0 and len(chunks) == 1):
                            mid_pair_cbs(hp)
                        emit_fill(1)
                        prev_ev = (pr, segs, pb2, ci == len(chunks) - 1)
            # tail: final AVs + normalize of the last pair, leftover
            # fillers (e.g. unconsumed outproj quanta), last outproj
            consume_prev()
            emit_fill(len(filler))
            while pending_fins:
                pending_fins.popleft()()
            so = state["pending_out"]
            for q in outproj_quanta(so, ot_sbs_by_s[so], mulss_by_s[so]):
                q()
            assert not filler

    nc.compile()
    return nc


def make_in_maps(x, m, w_qkv, w_out, nblk: int):
    """Host-side sharding/packing: core c = (batch c//2, head-group c%2)."""
    L = nblk * 128
    tri = np.where(
        np.arange(128)[None, :] >= np.arange(128)[:, None], 1.0, 0.0
    ).astype(NP_BF16)
    in_maps = []
    for c in range(N_CORES):
        b, g = divmod(c, 2)
        xt = np.ascontiguousarray(x[b].T[:, :L]).astype(NP_BF16).reshape(
            4, 128, L
        )
        wq = np.empty((128, 1024), np.float32)
        wk = np.empty((128, 1024), np.float32)
        for hp in range(2):
            for d4 in range(4):
                rows = slice(128 * d4, 128 * (d4 + 1))
                qcol = 256 * g + 128 * hp
                col = 128 * (4 * hp + d4)
                wq[:, col : col + 128] = w_qkv[rows, qcol : qcol + 128] * SCALE
                wk[:, col : col + 128] = w_qkv[rows, 512 + qcol : 512 + qcol + 128]
        wv = np.empty((128, 1024), np.float32)
        for d4 in range(4):
            wv[:, 256 * d4 : 256 * (d4 + 1)] = w_qkv[
                128 * d4 : 128 * (d4 + 1), 1024 + 256 * g : 1024 + 256 * (g + 1)
            ]
        wo = np.empty((128, 1024), np.float32)
        for hp in range(2):
            r0 = 256 * g + 128 * hp
            wo[:, 512 * hp : 512 * (hp + 1)] = w_out[r0 : r0 + 128, :]
        mp = np.ascontiguousarray(
            m[b, :L, 0].reshape(nblk, 128).T
        ).astype(np.float32)
        in_maps.append(
            {
                "xt": xt,
                "wq": wq.astype(NP_BF16),
                "wk": wk.astype(NP_BF16),
                "wv": wv.astype(NP_BF16),
                "wo": wo.astype(NP_BF16),
                "m": mp,
                "tri": tri,
            }
        )
    return in_maps


def postprocess(results, x, m, b_out):
    out = np.zeros((B, T, D), np.float32)
    for b in range(B):
        out[b] = results[2 * b]["out"].astype(np.float32) + results[
            2 * b + 1
        ]["out"].astype(np.float32)
    out += b_out[None, None, :].astype(np.float32) * m.astype(np.float32)
    return out


def kernel(x, m, w_qkv, w_out, b_out):
    lengths = m[:, :, 0].astype(np.int64).sum(axis=1)
    nblk = max(1, int(-(-lengths.max() // 128)))
    nc = build_nc(nblk)
    in_maps = make_in_maps(x, m, w_qkv, w_out, nblk)
    res = bass_utils.run_bass_kernel_spmd(nc, in_maps, core_ids=list(range(N_CORES)))
    return postprocess(res.results, x, m, b_out)
